# revision 13
# baseline (speedup 1.0000x reference)
"""Trainium2 Bass kernel for an attention block (QKV -> 16-head attention ->
out-proj -> residual + LayerNorm), distributed over 8 NeuronCores.

Sharding: core c handles batch b = c//2 and head-group g = c%2 (8 of 16
heads).  The pair (2b, 2b+1) jointly owns batch b; partial attention outputs
and head-averaged attention weights are combined with pairwise ReduceScatter
collectives, after which each core LayerNorms its half of the rows.

On-chip layouts (per core):
  - scores computed transposed: scoresT[k, q] = sum_d kT[d,k] qT[d,q]
  - exp on ScalarE (PSUM f32 -> SBUF bf16); softmax denominators via a
    ones-column appended to V in the ctx matmul (row 64 of ctxT_aug)
  - ctxT [din, q] feeds out-proj as the stationary operand, producing
    attn_out in natural [q, d] layout for the LayerNorm
  - attention-mean accumulated as acc[k, q] += expT_h * recipB_h with
    reciprocal rows broadcast across partitions by GpSimd; the accumulator
    ping-pongs between two buffers (in-place DVE adds are ~5x slower)
  - the first ReduceScatter is issued before the last pair's mean work so
    the collective overlaps compute
Host pre-transposes/casts weights (free) and reassembles output halves.
"""

import sys

sys.path.insert(0, "/opt/trn_rl_repo")

import numpy as np
import ml_dtypes

import concourse.bass as bass
import concourse.tile as tile
from concourse import bacc, mybir
from concourse.bass import ts

BF16 = mybir.dt.bfloat16
F32 = mybir.dt.float32
AX = mybir.AluOpType
AF = mybir.ActivationFunctionType

B, S, D = 4, 1024, 1024
H, HD = 16, 64
HG = H // 2          # heads per core = 8
N_CORES = 8
LN_EPS = 1e-5
SH = S // 2          # rows per core after reduce-scatter
GROUPS = [[0, 1], [2, 3], [4, 5], [6, 7]]


def _build(flags):
    ln_affine, bv_zero, bo_zero = flags
    nc = bacc.Bacc("TRN2", target_bir_lowering=False, debug=False, num_devices=N_CORES)

    io = {
        "xT": nc.declare_dram_parameter("xT", [D, S], BF16, isOutput=False),
        "xr": nc.declare_dram_parameter("xr", [SH, D], F32, isOutput=False),
        "wqkT": nc.declare_dram_parameter("wqkT", [D, 1024], BF16, isOutput=False),
        "wvT": nc.declare_dram_parameter("wvT", [D, 512], BF16, isOutput=False),
        "woutT": nc.declare_dram_parameter("woutT", [512, D], BF16, isOutput=False),
        "bqk": nc.declare_dram_parameter("bqk", [1024], F32, isOutput=False),
        "bv": nc.declare_dram_parameter("bv", [512], F32, isOutput=False),
        "bo": nc.declare_dram_parameter("bo", [D], F32, isOutput=False),
        "lnw": nc.declare_dram_parameter("lnw", [D], F32, isOutput=False),
        "lnb": nc.declare_dram_parameter("lnb", [D], F32, isOutput=False),
        "y": nc.declare_dram_parameter("y", [SH, D], F32, isOutput=True),
        "attn": nc.declare_dram_parameter("attn", [SH, S], BF16, isOutput=True),
        "ao_bounce": nc.dram_tensor("ao_bounce", [S, D], BF16),
        "ao_rs": nc.dram_tensor("ao_rs", [SH, D], BF16),
        "at_bounce": nc.dram_tensor("at_bounce", [S, S], BF16),
        "at_rs": nc.dram_tensor("at_rs", [SH, S], BF16),
    }

    with tile.TileContext(nc) as tc:
        _emit(tc, nc, io, ln_affine, bv_zero, bo_zero)
    nc.compile()
    return nc


def _emit(tc, nc, io, ln_affine, bv_zero, bo_zero):
    with tc.tile_pool(name="persist", bufs=1) as persist, \
         tc.tile_pool(name="consts", bufs=1) as consts:

        # ---------- persistent SBUF ----------
        woutT_sb = persist.tile([128, 4, D], BF16)
        qkT_sb = persist.tile([128, 8, S], BF16)       # j-tiles 0-3: qT, 4-7: kT
        v_sb = persist.tile([128, 8, HG, 65], BF16)    # [kt, head, dim(64)+ones]
        ctxT_sb = persist.tile([128, 4, S], BF16)      # [din-tile, q]
        acc_a = persist.tile([128, 8, S], BF16)        # mean acc ping
        acc_b = persist.tile([128, 8, S], BF16)        # mean acc pong

        for dt in range(4):
            nc.sync.dma_start(
                woutT_sb[:, dt, :],
                io["woutT"].ap().rearrange("(a p) d -> p a d", p=128)[:, dt, :])

        bqk_sb = consts.tile([128, 8], F32)
        nc.sync.dma_start(bqk_sb[:, :],
                          bass.AP(tensor=io["bqk"], offset=0, ap=[[1, 128], [128, 8]]))
        if not bv_zero:
            bvB = consts.tile([128, 8, 64], F32)
            nc.sync.dma_start(bvB[:, :, :],
                              bass.AP(tensor=io["bv"], offset=0,
                                      ap=[[0, 128], [64, 8], [1, 64]]))
        if not bo_zero:
            boB = consts.tile([128, D], F32)
            nc.sync.dma_start(boB[:, :],
                              bass.AP(tensor=io["bo"], offset=0, ap=[[0, 128], [1, D]]))

        nc.vector.memset(v_sb[:, :, :, 64:65], 1.0)

        # ---------- QKV + attention (scoped pools) ----------
        with tc.tile_pool(name="weights", bufs=1) as weights, \
             tc.tile_pool(name="expp", bufs=3) as exp_pool, \
             tc.tile_pool(name="stage", bufs=1) as stage_pool, \
             tc.tile_pool(name="scl", bufs=3) as scl_pool, \
             tc.tile_pool(name="rbp", bufs=3) as rb_pool, \
             tc.tile_pool(name="pbs", bufs=1) as pb_pool, \
             tc.tile_pool(name="ps_big", bufs=2, space="PSUM") as ps_big, \
             tc.tile_pool(name="ps_ctx", bufs=2, space="PSUM") as ps_ctx, \
             tc.tile_pool(name="ao", bufs=2) as ao_pool:

            xT_sb = weights.tile([128, 8, S], BF16)
            wqkT_sb = weights.tile([128, 8, 1024], BF16)
            wvT_sb = weights.tile([128, 8, 512], BF16)
            # per-tile DMAs so compute can start on the first slice
            for dt in range(8):
                nc.sync.dma_start(
                    wqkT_sb[:, dt, :],
                    io["wqkT"].ap().rearrange("(a p) j -> p a j", p=128)[:, dt, :])
                nc.scalar.dma_start(
                    xT_sb[:, dt, :],
                    io["xT"].ap().rearrange("(a p) s -> p a s", p=128)[:, dt, :])
                nc.sync.dma_start(
                    wvT_sb[:, dt, :],
                    io["wvT"].ap().rearrange("(a p) v -> p a v", p=128)[:, dt, :])

            def emit_qk(jt):
                ps = ps_big.tile([128, 1024], F32, tag="ps", name=f"psqk{jt}")
                for dt in range(8):
                    for n in range(2):
                        nc.tensor.matmul(
                            ps[:, ts(n, 512)],
                            lhsT=wqkT_sb[:, dt, ts(jt, 128)],
                            rhs=xT_sb[:, dt, ts(n, 512)],
                            start=(dt == 0), stop=(dt == 7),
                        )
                # eviction with fused per-partition bias add (ScalarE)
                nc.scalar.activation(out=qkT_sb[:, jt, :], in_=ps[:, :],
                                     func=AF.Identity,
                                     bias=bqk_sb[:, jt : jt + 1], scale=1.0)

            def emit_v(st):
                ps = ps_big.tile([128, 1024], F32, tag="ps", name=f"psv{st}")
                for dt in range(8):
                    nc.tensor.matmul(
                        ps[:, 0:512],
                        lhsT=xT_sb[:, dt, ts(st, 128)],
                        rhs=wvT_sb[:, dt, :],
                        start=(dt == 0), stop=(dt == 7),
                    )
                if bv_zero:
                    nc.vector.tensor_copy(
                        v_sb[:, st, :, 0:64],
                        ps[:, 0:512].rearrange("p (h d) -> p h d", h=HG))
                else:
                    nc.vector.scalar_tensor_tensor(
                        out=v_sb[:, st, :, 0:64],
                        in0=ps[:, 0:512].rearrange("p (h d) -> p h d", h=HG),
                        scalar=1.0, in1=bvB[:, :, :],
                        op0=AX.bypass, op1=AX.add)

            def emit_pair_compute(hp):
                h0, h1 = 2 * hp, 2 * hp + 1
                exp_t = {h: exp_pool.tile([128, 8, S], BF16, tag="exp", name=f"exp{h}")
                         for h in (h0, h1)}
                pctx = {h: ps_ctx.tile([65, 1024], F32, tag="ctx", name=f"pctx{h}")
                        for h in (h0, h1)}
                for kt in range(8):
                    for i, h in enumerate((h0, h1)):
                        lo = 64 * i
                        ps = ps_big.tile([128, 1024], F32, tag="ps", name=f"pssc{h}_{kt}")
                        for n in range(2):
                            nc.tensor.matmul(
                                ps[:, ts(n, 512)],
                                lhsT=qkT_sb[lo : lo + 64, 4 + hp, ts(kt, 128)],
                                rhs=qkT_sb[lo : lo + 64, hp, ts(n, 512)],
                                start=True, stop=True,
                            )
                        nc.scalar.activation(out=exp_t[h][:, kt, :], in_=ps[:, :],
                                             func=AF.Exp)
                        for n in range(2):
                            nc.tensor.matmul(
                                pctx[h][:, ts(n, 512)],
                                lhsT=v_sb[:, kt, h, :],
                                rhs=exp_t[h][:, kt, ts(n, 512)],
                                start=(kt == 0), stop=(kt == 7),
                                skip_group_check=True,
                            )
                pair_sums = pb_pool.tile([2, S], F32, tag="psums", name=f"psums{hp}")
                pair_recip = pb_pool.tile([2, S], F32, tag="precip", name=f"precip{hp}")
                pair_rbf = pb_pool.tile([2, S], BF16, tag="prbf", name=f"prbf{hp}")
                rB = {}
                for i, h in enumerate((h0, h1)):
                    if i == 0:
                        nc.vector.tensor_copy(ctxT_sb[0:64, hp, :], pctx[h][0:64, :])
                    else:
                        odd_stage = stage_pool.tile([64, S], BF16, tag="odd")
                        nc.vector.tensor_copy(odd_stage[:, :], pctx[h][0:64, :])
                        nc.sync.dma_start(ctxT_sb[64:128, hp, :], odd_stage[:, :])
                    sstage = stage_pool.tile([65, S], F32, tag="sum")
                    nc.scalar.copy(sstage[64:65, :], pctx[h][64:65, :])
                    nc.sync.dma_start(pair_sums[i : i + 1, :], sstage[64:65, :])
                # recip rows: 1/(16*sum); wout is pre-scaled by 16 on the host
                nc.vector.reciprocal_approx_fast(out=pair_recip[:, :],
                                                 in_=pair_sums[:, :])
                nc.vector.tensor_scalar(out=pair_rbf[:, :], in0=pair_recip[:, :],
                                        scalar1=1.0 / 16.0, scalar2=None, op0=AX.mult)
                pb_stage = pb_pool.tile([1, 2, S], BF16, tag="pb")
                nc.sync.dma_start(pb_stage[0:1, :, :], pair_rbf[:, :])
                for i, h in enumerate((h0, h1)):
                    rB[h] = rb_pool.tile([128, S], BF16, tag="rb", name=f"rB{h}")
                    nc.gpsimd.partition_broadcast(rB[h][:, :], pb_stage[0:1, i, :])
                # normalize ctxT columns
                nc.gpsimd.tensor_tensor(out=ctxT_sb[0:64, hp, :],
                                         in0=ctxT_sb[0:64, hp, :],
                                         in1=rB[h0][0:64, :], op=AX.mult)
                nc.gpsimd.tensor_tensor(out=ctxT_sb[64:128, hp, :],
                                         in0=ctxT_sb[64:128, hp, :],
                                         in1=rB[h1][64:128, :], op=AX.mult)
                return exp_t, rB

            def emit_pair_mean(hp, exp_t, rB):
                # acc chain with ping-pong: in-place DVE adds run ~5x slower,
                # so each add writes the other buffer; final lands in acc_b
                for h in (2 * hp, 2 * hp + 1):
                    for kt in range(8):
                        meng = nc.gpsimd if kt % 2 == 0 else nc.vector
                        if h == 0:
                            meng.tensor_tensor(out=acc_a[:, kt, :],
                                               in0=exp_t[h][:, kt, :],
                                               in1=rB[h][:, :], op=AX.mult)
                        else:
                            src = acc_a if h % 2 == 1 else acc_b
                            dst = acc_b if h % 2 == 1 else acc_a
                            scl = scl_pool.tile([128, S], BF16, tag="scl")
                            meng.tensor_tensor(out=scl[:, :],
                                               in0=exp_t[h][:, kt, :],
                                               in1=rB[h][:, :], op=AX.mult)
                            nc.vector.tensor_tensor(out=dst[:, kt, :],
                                                    in0=src[:, kt, :],
                                                    in1=scl[:, :], op=AX.add)

            def emit_outproj():
                for qt in range(8):
                    ps = ps_big.tile([128, 1024], F32, tag="ps", name=f"psao{qt}")
                    for dt in range(4):
                        for n in range(2):
                            nc.tensor.matmul(
                                ps[:, ts(n, 512)],
                                lhsT=ctxT_sb[:, dt, ts(qt, 128)],
                                rhs=woutT_sb[:, dt, ts(n, 512)],
                                start=(dt == 0), stop=(dt == 3),
                            )
                    ao_sb = ao_pool.tile([128, D], BF16, tag="aosb")
                    if bo_zero:
                        nc.scalar.copy(ao_sb[:, :], ps[:, :])
                    else:
                        nc.vector.scalar_tensor_tensor(
                            out=ao_sb[:, :], in0=ps[:, :], scalar=1.0, in1=boB[:, :],
                            op0=AX.bypass, op1=AX.add)
                    nc.sync.dma_start(io["ao_bounce"][ts(qt, 128), :], ao_sb[:, :])

            for jt in (0, 4, 1, 5):
                emit_qk(jt)
            for st in range(8):
                emit_v(st)
            e0, r0 = emit_pair_compute(0)
            emit_pair_mean(0, e0, r0)
            emit_qk(2)
            emit_qk(6)
            e1, r1 = emit_pair_compute(1)
            emit_pair_mean(1, e1, r1)
            emit_qk(3)
            emit_qk(7)
            e2, r2 = emit_pair_compute(2)
            emit_pair_mean(2, e2, r2)
            e3, r3 = emit_pair_compute(3)

            # out-proj + first collective BEFORE the last pair's mean work so
            # the ReduceScatter overlaps with it
            emit_outproj()
            nc.gpsimd.collective_compute(
                "ReduceScatter", AX.add, replica_groups=GROUPS,
                ins=[io["ao_bounce"].ap().opt()], outs=[io["ao_rs"].ap().opt()],
            )
            # pair-3 mean kt-major, with the attention collective issued in
            # halves as soon as each half's accumulator is final
            for half in range(2):
                for kt in range(4 * half, 4 * half + 4):
                    for h in (6, 7):
                        meng = nc.gpsimd if kt % 2 == 0 else nc.vector
                        src_t = acc_a if h % 2 == 1 else acc_b
                        dst_t = acc_b if h % 2 == 1 else acc_a
                        scl = scl_pool.tile([128, S], BF16, tag="scl")
                        meng.tensor_tensor(out=scl[:, :],
                                           in0=e3[h][:, kt, :],
                                           in1=r3[h][:, :], op=AX.mult)
                        nc.vector.tensor_tensor(out=dst_t[:, kt, :],
                                                in0=src_t[:, kt, :],
                                                in1=scl[:, :], op=AX.add)
                for kt in range(4 * half, 4 * half + 4):
                    nc.sync.dma_start(io["at_bounce"][ts(kt, 128), :],
                                      acc_b[:, kt, :])
                nc.gpsimd.collective_compute(
                    "ReduceScatter", AX.add, replica_groups=GROUPS,
                    ins=[io["at_bounce"][512 * half : 512 * half + 512, :].opt()],
                    outs=[io["at_rs"][256 * half : 256 * half + 256, :].opt()],
                )
            nc.sync.dma_start(io["attn"].ap(), io["at_rs"].ap())

        # ---------- residual + LayerNorm on our half ----------
        with tc.tile_pool(name="ln", bufs=1) as ln_pool:
            xao = ln_pool.tile([128, 4, D], F32)
            xres = ln_pool.tile([128, 4, D], F32)
            aohalf = ln_pool.tile([128, 4, D], BF16)
            nc.sync.dma_start(aohalf[:, :, :],
                              io["ao_rs"].ap().rearrange("(a p) d -> p a d", p=128))
            nc.sync.dma_start(xres[:, :, :],
                              io["xr"].ap().rearrange("(a p) d -> p a d", p=128))
            stats = ln_pool.tile([128, 4, 2, 6], F32)
            mv = ln_pool.tile([128, 4, 2], F32)
            for a in range(4):
                nc.vector.tensor_tensor(out=xao[:, a, :], in0=xres[:, a, :],
                                        in1=aohalf[:, a, :], op=AX.add)
                for half in range(2):
                    nc.vector.bn_stats(out=stats[:, a, half, :],
                                       in_=xao[:, a, ts(half, 512)])
                nc.vector.bn_aggr(out=mv[:, a, :], in_=stats[:, a, :, :])
            eps_sb = ln_pool.tile([128, 1], F32)
            nc.vector.memset(eps_sb[:, :], LN_EPS)
            rstd = ln_pool.tile([128, 4], F32)
            nmr = ln_pool.tile([128, 4], F32)
            nc.scalar.activation(out=rstd[:, :], in_=mv[:, :, 1], func=AF.Sqrt,
                                 bias=eps_sb[:, 0:1], scale=1.0)
            nc.vector.reciprocal(out=rstd[:, :], in_=rstd[:, :])
            nc.vector.scalar_tensor_tensor(
                out=nmr[:, :], in0=mv[:, :, 0], scalar=-1.0, in1=rstd[:, :],
                op0=AX.mult, op1=AX.mult)
            if ln_affine:
                lnwB = ln_pool.tile([128, D], F32)
                lnbB = ln_pool.tile([128, D], F32)
                nc.sync.dma_start(lnwB[:, :],
                                  bass.AP(tensor=io["lnw"], offset=0,
                                          ap=[[0, 128], [1, D]]))
                nc.sync.dma_start(lnbB[:, :],
                                  bass.AP(tensor=io["lnb"], offset=0,
                                          ap=[[0, 128], [1, D]]))
            for a in range(4):
                nc.scalar.activation(out=xao[:, a, :], in_=xao[:, a, :],
                                     func=AF.Identity,
                                     bias=nmr[:, a : a + 1], scale=rstd[:, a : a + 1])
                if ln_affine:
                    nc.vector.tensor_tensor(out=xao[:, a, :], in0=xao[:, a, :],
                                            in1=lnwB[:, :], op=AX.mult)
                    nc.vector.tensor_tensor(out=xao[:, a, :], in0=xao[:, a, :],
                                            in1=lnbB[:, :], op=AX.add)
                nc.sync.dma_start(
                    io["y"].ap().rearrange("(a p) d -> p a d", p=128)[:, a, :],
                    xao[:, a, :])


_NC_CACHE = {}


def _get_nc(flags):
    if flags not in _NC_CACHE:
        _NC_CACHE[flags] = _build(flags)
    return _NC_CACHE[flags]


def _prep_in_maps(x, w_qkv, b_qkv, w_out, b_out, ln_w, ln_b):
    bf = ml_dtypes.bfloat16
    s_q = 1.0 / np.sqrt(HD)
    wq = w_qkv[0:D, :]
    wk = w_qkv[D : 2 * D, :]
    wv = w_qkv[2 * D : 3 * D, :]
    bq, bk, bvv = b_qkv[0:D], b_qkv[D : 2 * D], b_qkv[2 * D : 3 * D]
    woutT_full = np.ascontiguousarray(w_out.T) * 16.0  # undo the 1/16 in recip rows

    in_maps = []
    for c in range(N_CORES):
        b, g = c // 2, c % 2
        rows = slice(g * 512, (g + 1) * 512)
        wqg = (wq[rows, :] * s_q).astype(bf)
        wkg = wk[rows, :].astype(bf)
        wqkT = np.ascontiguousarray(np.concatenate([wqg, wkg], axis=0).T.astype(bf))
        xb = x[b]
        half = slice(g * SH, g * SH + SH)
        in_maps.append(
            {
                "xT": np.ascontiguousarray(xb.T.astype(bf)),
                "xr": np.ascontiguousarray(xb[half, :]).astype(np.float32),
                "wqkT": wqkT,
                "wvT": np.ascontiguousarray(wv[rows, :].T.astype(bf)),
                "woutT": np.ascontiguousarray(woutT_full[rows, :].astype(bf)),
                "bqk": np.concatenate([bq[rows] * s_q, bk[rows]]).astype(np.float32),
                "bv": bvv[rows].astype(np.float32),
                "bo": (b_out * 0.5).astype(np.float32),
                "lnw": ln_w.astype(np.float32),
                "lnb": ln_b.astype(np.float32),
            }
        )
    return in_maps


def _assemble(results):
    y = np.empty((B, S, D), dtype=np.float32)
    attn = np.empty((B, S, S), dtype=np.float32)
    for b in range(B):
        even, odd = results[2 * b], results[2 * b + 1]
        y[b, 0:SH, :] = even["y"]
        y[b, SH:S, :] = odd["y"]
        # chunked RS: each half-collective scatters its chunk across the pair
        ev, od = even["attn"].astype(np.float32), odd["attn"].astype(np.float32)
        at = np.concatenate([ev[0:256], od[0:256], ev[256:512], od[256:512]], axis=0)
        attn[b] = at.T
    return y, attn


def _flags(b_qkv, b_out, ln_w, ln_b):
    ln_affine = not (np.all(ln_w == 1.0) and np.all(ln_b == 0.0))
    bv_zero = bool(np.all(b_qkv[2 * D : 3 * D] == 0.0))
    bo_zero = bool(np.all(b_out == 0.0))
    return (ln_affine, bv_zero, bo_zero)


def kernel(x, w_qkv, b_qkv, w_out, b_out, ln_w, ln_b, _trace=False):
    from concourse.bass_utils import run_bass_kernel_spmd

    x = np.asarray(x, dtype=np.float32)
    w_qkv = np.asarray(w_qkv, dtype=np.float32)
    b_qkv = np.asarray(b_qkv, dtype=np.float32)
    w_out = np.asarray(w_out, dtype=np.float32)
    b_out = np.asarray(b_out, dtype=np.float32)
    ln_w = np.asarray(ln_w, dtype=np.float32)
    ln_b = np.asarray(ln_b, dtype=np.float32)

    nc = _get_nc(_flags(b_qkv, b_out, ln_w, ln_b))
    in_maps = _prep_in_maps(x, w_qkv, b_qkv, w_out, b_out, ln_w, ln_b)
    res = run_bass_kernel_spmd(nc, in_maps, core_ids=list(range(N_CORES)), trace=_trace)
    out = _assemble(res.results)
    if _trace:
        kernel.last_exec_time_ns = res.exec_time_ns
    return out


# ---- simulation entry for development (not used by the harness) ----
def simulate(x, w_qkv, b_qkv, w_out, b_out, ln_w, ln_b):
    from concourse import bass_interp

    nc = _get_nc(_flags(np.asarray(b_qkv), np.asarray(b_out),
                        np.asarray(ln_w), np.asarray(ln_b)))
    in_maps = _prep_in_maps(
        np.asarray(x, np.float32), np.asarray(w_qkv, np.float32),
        np.asarray(b_qkv, np.float32), np.asarray(w_out, np.float32),
        np.asarray(b_out, np.float32), np.asarray(ln_w, np.float32),
        np.asarray(ln_b, np.float32),
    )
    sim = bass_interp.MultiCoreSim(nc, N_CORES)
    for i in range(N_CORES):
        for k, vv in in_maps[i].items():
            sim.cores[i].tensor(k)[:] = vv
    sim.simulate()
    results = [
        {k: np.array(sim.cores[i].mem_tensor(k)) for k in ("y", "attn")}
        for i in range(N_CORES)
    ]
    return _assemble(results)


# revision 14
# speedup vs baseline: 1.1898x; 1.1898x over previous
"""Trainium2 Bass kernel for an attention block (QKV -> 16-head attention ->
out-proj -> residual + LayerNorm), distributed over 8 NeuronCores.

Sharding: core c handles batch b = c//2 and head-group g = c%2 (8 of 16
heads).  The pair (2b, 2b+1) jointly owns batch b; partial attention outputs
and head-averaged attention weights are combined with pairwise ReduceScatter
collectives, after which each core LayerNorms its half of the rows.

On-chip layouts (per core):
  - scores computed transposed: scoresT[k, q] = sum_d kT[d,k] qT[d,q]
  - exp on ScalarE (PSUM f32 -> SBUF bf16); softmax denominators via a
    ones-column appended to V in the ctx matmul (row 64 of ctxT_aug)
  - ctxT [din, q] feeds out-proj as the stationary operand, producing
    attn_out in natural [q, d] layout for the LayerNorm
  - attention-mean accumulated as acc[k, q] += expT_h * recipB_h with
    reciprocal rows broadcast across partitions by GpSimd; the accumulator
    ping-pongs between two buffers (in-place DVE adds are ~5x slower)
  - the first ReduceScatter is issued before the last pair's mean work so
    the collective overlaps compute
Host pre-transposes/casts weights (free) and reassembles output halves.
"""

import sys

sys.path.insert(0, "/opt/trn_rl_repo")

import numpy as np
import ml_dtypes

import concourse.bass as bass
import concourse.tile as tile
from concourse import bacc, mybir
from concourse.bass import ts

BF16 = mybir.dt.bfloat16
F32 = mybir.dt.float32
AX = mybir.AluOpType
AF = mybir.ActivationFunctionType

B, S, D = 4, 1024, 1024
H, HD = 16, 64
HG = H // 2          # heads per core = 8
N_CORES = 8
LN_EPS = 1e-5
SH = S // 2          # rows per core after reduce-scatter
GROUPS = [[0, 1], [2, 3], [4, 5], [6, 7]]


def _build(flags):
    ln_affine, bv_zero, bo_zero = flags
    nc = bacc.Bacc("TRN2", target_bir_lowering=False, debug=False, num_devices=N_CORES)

    io = {
        "xT": nc.declare_dram_parameter("xT", [D, S], BF16, isOutput=False),
        "xr": nc.declare_dram_parameter("xr", [SH, D], F32, isOutput=False),
        "wqkT": nc.declare_dram_parameter("wqkT", [D, 1024], BF16, isOutput=False),
        "wvT": nc.declare_dram_parameter("wvT", [D, 512], BF16, isOutput=False),
        "woutT": nc.declare_dram_parameter("woutT", [512, D], BF16, isOutput=False),
        "bqk": nc.declare_dram_parameter("bqk", [1024], F32, isOutput=False),
        "bv": nc.declare_dram_parameter("bv", [512], F32, isOutput=False),
        "bo": nc.declare_dram_parameter("bo", [D], F32, isOutput=False),
        "lnw": nc.declare_dram_parameter("lnw", [D], F32, isOutput=False),
        "lnb": nc.declare_dram_parameter("lnb", [D], F32, isOutput=False),
        "y": nc.declare_dram_parameter("y", [SH, D], F32, isOutput=True),
        "attn": nc.declare_dram_parameter("attn", [SH, S], BF16, isOutput=True),
        "ao_bounce": nc.dram_tensor("ao_bounce", [S, D], BF16),
        "ao_rs": nc.dram_tensor("ao_rs", [SH, D], BF16),
        "at_bounce": nc.dram_tensor("at_bounce", [S, S], BF16),
        "at_rs": nc.dram_tensor("at_rs", [SH, S], BF16),
    }

    with tile.TileContext(nc) as tc:
        _emit(tc, nc, io, ln_affine, bv_zero, bo_zero)
    nc.compile()
    return nc


def _emit(tc, nc, io, ln_affine, bv_zero, bo_zero):
    with tc.tile_pool(name="persist", bufs=1) as persist, \
         tc.tile_pool(name="consts", bufs=1) as consts:

        # ---------- persistent SBUF ----------
        woutT_sb = persist.tile([128, 4, D], BF16)
        qkT_sb = persist.tile([128, 8, S], BF16)       # j-tiles 0-3: qT, 4-7: kT
        v_sb = persist.tile([128, 8, HG, 65], BF16)    # [kt, head, dim(64)+ones]
        ctxT_sb = persist.tile([128, 4, S], BF16)      # [din-tile, q]
        acc_a = persist.tile([128, 8, S], BF16)        # mean acc ping
        acc_b = persist.tile([128, 8, S], BF16)        # mean acc pong

        for dt in range(4):
            nc.sync.dma_start(
                woutT_sb[:, dt, :],
                io["woutT"].ap().rearrange("(a p) d -> p a d", p=128)[:, dt, :])

        bqk_sb = consts.tile([128, 8], F32)
        nc.sync.dma_start(bqk_sb[:, :],
                          bass.AP(tensor=io["bqk"], offset=0, ap=[[1, 128], [128, 8]]))
        if not bv_zero:
            bvB = consts.tile([128, 8, 64], F32)
            nc.sync.dma_start(bvB[:, :, :],
                              bass.AP(tensor=io["bv"], offset=0,
                                      ap=[[0, 128], [64, 8], [1, 64]]))
        if not bo_zero:
            boB = consts.tile([128, D], F32)
            nc.sync.dma_start(boB[:, :],
                              bass.AP(tensor=io["bo"], offset=0, ap=[[0, 128], [1, D]]))

        nc.vector.memset(v_sb[:, :, :, 64:65], 1.0)

        # ---------- QKV + attention (scoped pools) ----------
        with tc.tile_pool(name="weights", bufs=1) as weights, \
             tc.tile_pool(name="expp", bufs=3) as exp_pool, \
             tc.tile_pool(name="stage", bufs=1) as stage_pool, \
             tc.tile_pool(name="scl", bufs=3) as scl_pool, \
             tc.tile_pool(name="rbp", bufs=3) as rb_pool, \
             tc.tile_pool(name="pbs", bufs=1) as pb_pool, \
             tc.tile_pool(name="ps_big", bufs=2, space="PSUM") as ps_big, \
             tc.tile_pool(name="ps_ctx", bufs=2, space="PSUM") as ps_ctx, \
             tc.tile_pool(name="ao", bufs=2) as ao_pool:

            xT_sb = weights.tile([128, 8, S], BF16)
            wqkT_sb = weights.tile([128, 8, 1024], BF16)
            wvT_sb = weights.tile([128, 8, 512], BF16)
            # per-tile DMAs so compute can start on the first slice
            for dt in range(8):
                nc.sync.dma_start(
                    wqkT_sb[:, dt, :],
                    io["wqkT"].ap().rearrange("(a p) j -> p a j", p=128)[:, dt, :])
                nc.sync.dma_start(
                    xT_sb[:, dt, :],
                    io["xT"].ap().rearrange("(a p) s -> p a s", p=128)[:, dt, :])
                nc.sync.dma_start(
                    wvT_sb[:, dt, :],
                    io["wvT"].ap().rearrange("(a p) v -> p a v", p=128)[:, dt, :])

            def emit_qk(jt):
                ps = ps_big.tile([128, 1024], F32, tag="ps", name=f"psqk{jt}")
                for dt in range(8):
                    for n in range(2):
                        nc.tensor.matmul(
                            ps[:, ts(n, 512)],
                            lhsT=wqkT_sb[:, dt, ts(jt, 128)],
                            rhs=xT_sb[:, dt, ts(n, 512)],
                            start=(dt == 0), stop=(dt == 7),
                        )
                # eviction with fused per-partition bias add (ScalarE)
                nc.scalar.activation(out=qkT_sb[:, jt, :], in_=ps[:, :],
                                     func=AF.Identity,
                                     bias=bqk_sb[:, jt : jt + 1], scale=1.0)

            def emit_v(st):
                ps = ps_big.tile([128, 1024], F32, tag="ps", name=f"psv{st}")
                for dt in range(8):
                    nc.tensor.matmul(
                        ps[:, 0:512],
                        lhsT=xT_sb[:, dt, ts(st, 128)],
                        rhs=wvT_sb[:, dt, :],
                        start=(dt == 0), stop=(dt == 7),
                    )
                if bv_zero:
                    nc.vector.tensor_copy(
                        v_sb[:, st, :, 0:64],
                        ps[:, 0:512].rearrange("p (h d) -> p h d", h=HG))
                else:
                    nc.vector.scalar_tensor_tensor(
                        out=v_sb[:, st, :, 0:64],
                        in0=ps[:, 0:512].rearrange("p (h d) -> p h d", h=HG),
                        scalar=1.0, in1=bvB[:, :, :],
                        op0=AX.bypass, op1=AX.add)

            def emit_pair_compute(hp):
                h0, h1 = 2 * hp, 2 * hp + 1
                exp_t = {h: exp_pool.tile([128, 8, S], BF16, tag="exp", name=f"exp{h}")
                         for h in (h0, h1)}
                pctx = {h: ps_ctx.tile([65, 1024], F32, tag="ctx", name=f"pctx{h}")
                        for h in (h0, h1)}
                for kt in range(8):
                    for i, h in enumerate((h0, h1)):
                        lo = 64 * i
                        ps = ps_big.tile([128, 1024], F32, tag="ps", name=f"pssc{h}_{kt}")
                        for n in range(2):
                            nc.tensor.matmul(
                                ps[:, ts(n, 512)],
                                lhsT=qkT_sb[lo : lo + 64, 4 + hp, ts(kt, 128)],
                                rhs=qkT_sb[lo : lo + 64, hp, ts(n, 512)],
                                start=True, stop=True,
                            )
                        nc.scalar.activation(out=exp_t[h][:, kt, :], in_=ps[:, :],
                                             func=AF.Exp)
                        for n in range(2):
                            nc.tensor.matmul(
                                pctx[h][:, ts(n, 512)],
                                lhsT=v_sb[:, kt, h, :],
                                rhs=exp_t[h][:, kt, ts(n, 512)],
                                start=(kt == 0), stop=(kt == 7),
                                skip_group_check=True,
                            )
                pair_sums = pb_pool.tile([2, S], F32, tag="psums", name=f"psums{hp}")
                pair_recip = pb_pool.tile([2, S], F32, tag="precip", name=f"precip{hp}")
                pair_rbf = pb_pool.tile([2, S], BF16, tag="prbf", name=f"prbf{hp}")
                rB = {}
                for i, h in enumerate((h0, h1)):
                    if i == 0:
                        nc.vector.tensor_copy(ctxT_sb[0:64, hp, :], pctx[h][0:64, :])
                    else:
                        odd_stage = stage_pool.tile([64, S], BF16, tag="odd")
                        nc.vector.tensor_copy(odd_stage[:, :], pctx[h][0:64, :])
                        nc.sync.dma_start(ctxT_sb[64:128, hp, :], odd_stage[:, :])
                    sstage = stage_pool.tile([65, S], F32, tag="sum")
                    nc.scalar.copy(sstage[64:65, :], pctx[h][64:65, :])
                    nc.sync.dma_start(pair_sums[i : i + 1, :], sstage[64:65, :])
                # recip rows: 1/(16*sum); wout is pre-scaled by 16 on the host
                nc.vector.reciprocal_approx_fast(out=pair_recip[:, :],
                                                 in_=pair_sums[:, :])
                nc.vector.tensor_scalar(out=pair_rbf[:, :], in0=pair_recip[:, :],
                                        scalar1=1.0 / 16.0, scalar2=None, op0=AX.mult)
                pb_stage = pb_pool.tile([1, 2, S], BF16, tag="pb")
                nc.sync.dma_start(pb_stage[0:1, :, :], pair_rbf[:, :])
                for i, h in enumerate((h0, h1)):
                    rB[h] = rb_pool.tile([128, S], BF16, tag="rb", name=f"rB{h}")
                    nc.gpsimd.partition_broadcast(rB[h][:, :], pb_stage[0:1, i, :])
                # normalize ctxT columns
                nc.vector.tensor_tensor(out=ctxT_sb[0:64, hp, :],
                                        in0=ctxT_sb[0:64, hp, :],
                                        in1=rB[h0][0:64, :], op=AX.mult)
                nc.vector.tensor_tensor(out=ctxT_sb[64:128, hp, :],
                                        in0=ctxT_sb[64:128, hp, :],
                                        in1=rB[h1][64:128, :], op=AX.mult)
                return exp_t, rB

            def emit_pair_mean(hp, exp_t, rB):
                # acc chain with ping-pong: in-place DVE adds run ~5x slower,
                # so each add writes the other buffer; final lands in acc_b
                for h in (2 * hp, 2 * hp + 1):
                    for kt in range(8):
                        if h == 0:
                            nc.vector.tensor_tensor(out=acc_a[:, kt, :],
                                                    in0=exp_t[h][:, kt, :],
                                                    in1=rB[h][:, :], op=AX.mult)
                        else:
                            src = acc_a if h % 2 == 1 else acc_b
                            dst = acc_b if h % 2 == 1 else acc_a
                            scl = scl_pool.tile([128, S], BF16, tag="scl")
                            nc.vector.tensor_tensor(out=scl[:, :],
                                                    in0=exp_t[h][:, kt, :],
                                                    in1=rB[h][:, :], op=AX.mult)
                            nc.vector.tensor_tensor(out=dst[:, kt, :],
                                                    in0=src[:, kt, :],
                                                    in1=scl[:, :], op=AX.add)

            def emit_outproj():
                for qt in range(8):
                    ps = ps_big.tile([128, 1024], F32, tag="ps", name=f"psao{qt}")
                    for dt in range(4):
                        for n in range(2):
                            nc.tensor.matmul(
                                ps[:, ts(n, 512)],
                                lhsT=ctxT_sb[:, dt, ts(qt, 128)],
                                rhs=woutT_sb[:, dt, ts(n, 512)],
                                start=(dt == 0), stop=(dt == 3),
                            )
                    ao_sb = ao_pool.tile([128, D], BF16, tag="aosb")
                    if bo_zero:
                        nc.scalar.copy(ao_sb[:, :], ps[:, :])
                    else:
                        nc.vector.scalar_tensor_tensor(
                            out=ao_sb[:, :], in0=ps[:, :], scalar=1.0, in1=boB[:, :],
                            op0=AX.bypass, op1=AX.add)
                    nc.sync.dma_start(io["ao_bounce"][ts(qt, 128), :], ao_sb[:, :])

            for jt in (0, 4, 1, 5):
                emit_qk(jt)
            for st in range(8):
                emit_v(st)
            e0, r0 = emit_pair_compute(0)
            emit_pair_mean(0, e0, r0)
            emit_qk(2)
            emit_qk(6)
            e1, r1 = emit_pair_compute(1)
            emit_pair_mean(1, e1, r1)
            emit_qk(3)
            emit_qk(7)
            e2, r2 = emit_pair_compute(2)
            emit_pair_mean(2, e2, r2)
            e3, r3 = emit_pair_compute(3)

            # out-proj + first collective BEFORE the last pair's mean work so
            # the ReduceScatter overlaps with it
            emit_outproj()
            nc.gpsimd.collective_compute(
                "ReduceScatter", AX.add, replica_groups=GROUPS,
                ins=[io["ao_bounce"].ap().opt()], outs=[io["ao_rs"].ap().opt()],
            )
            # pair-3 mean kt-major, with the attention collective issued in
            # halves as soon as each half's accumulator is final
            for half in range(2):
                for kt in range(4 * half, 4 * half + 4):
                    for h in (6, 7):
                        src_t = acc_a if h % 2 == 1 else acc_b
                        dst_t = acc_b if h % 2 == 1 else acc_a
                        scl = scl_pool.tile([128, S], BF16, tag="scl")
                        nc.vector.tensor_tensor(out=scl[:, :],
                                                in0=e3[h][:, kt, :],
                                                in1=r3[h][:, :], op=AX.mult)
                        nc.vector.tensor_tensor(out=dst_t[:, kt, :],
                                                in0=src_t[:, kt, :],
                                                in1=scl[:, :], op=AX.add)
                for kt in range(4 * half, 4 * half + 4):
                    nc.sync.dma_start(io["at_bounce"][ts(kt, 128), :],
                                      acc_b[:, kt, :])
                nc.gpsimd.collective_compute(
                    "ReduceScatter", AX.add, replica_groups=GROUPS,
                    ins=[io["at_bounce"][512 * half : 512 * half + 512, :].opt()],
                    outs=[io["at_rs"][256 * half : 256 * half + 256, :].opt()],
                )
            nc.sync.dma_start(io["attn"].ap(), io["at_rs"].ap())

        # ---------- residual + LayerNorm on our half ----------
        with tc.tile_pool(name="ln", bufs=1) as ln_pool:
            xao = ln_pool.tile([128, 4, D], F32)
            xres = ln_pool.tile([128, 4, D], F32)
            aohalf = ln_pool.tile([128, 4, D], BF16)
            nc.sync.dma_start(aohalf[:, :, :],
                              io["ao_rs"].ap().rearrange("(a p) d -> p a d", p=128))
            nc.sync.dma_start(xres[:, :, :],
                              io["xr"].ap().rearrange("(a p) d -> p a d", p=128))
            stats = ln_pool.tile([128, 4, 2, 6], F32)
            mv = ln_pool.tile([128, 4, 2], F32)
            for a in range(4):
                nc.vector.tensor_tensor(out=xao[:, a, :], in0=xres[:, a, :],
                                        in1=aohalf[:, a, :], op=AX.add)
                for half in range(2):
                    nc.vector.bn_stats(out=stats[:, a, half, :],
                                       in_=xao[:, a, ts(half, 512)])
                nc.vector.bn_aggr(out=mv[:, a, :], in_=stats[:, a, :, :])
            eps_sb = ln_pool.tile([128, 1], F32)
            nc.vector.memset(eps_sb[:, :], LN_EPS)
            rstd = ln_pool.tile([128, 4], F32)
            nmr = ln_pool.tile([128, 4], F32)
            nc.scalar.activation(out=rstd[:, :], in_=mv[:, :, 1], func=AF.Sqrt,
                                 bias=eps_sb[:, 0:1], scale=1.0)
            nc.vector.reciprocal(out=rstd[:, :], in_=rstd[:, :])
            nc.vector.scalar_tensor_tensor(
                out=nmr[:, :], in0=mv[:, :, 0], scalar=-1.0, in1=rstd[:, :],
                op0=AX.mult, op1=AX.mult)
            if ln_affine:
                lnwB = ln_pool.tile([128, D], F32)
                lnbB = ln_pool.tile([128, D], F32)
                nc.sync.dma_start(lnwB[:, :],
                                  bass.AP(tensor=io["lnw"], offset=0,
                                          ap=[[0, 128], [1, D]]))
                nc.sync.dma_start(lnbB[:, :],
                                  bass.AP(tensor=io["lnb"], offset=0,
                                          ap=[[0, 128], [1, D]]))
            for a in range(4):
                nc.scalar.activation(out=xao[:, a, :], in_=xao[:, a, :],
                                     func=AF.Identity,
                                     bias=nmr[:, a : a + 1], scale=rstd[:, a : a + 1])
                if ln_affine:
                    nc.vector.tensor_tensor(out=xao[:, a, :], in0=xao[:, a, :],
                                            in1=lnwB[:, :], op=AX.mult)
                    nc.vector.tensor_tensor(out=xao[:, a, :], in0=xao[:, a, :],
                                            in1=lnbB[:, :], op=AX.add)
                nc.sync.dma_start(
                    io["y"].ap().rearrange("(a p) d -> p a d", p=128)[:, a, :],
                    xao[:, a, :])


_NC_CACHE = {}


def _get_nc(flags):
    if flags not in _NC_CACHE:
        _NC_CACHE[flags] = _build(flags)
    return _NC_CACHE[flags]


def _prep_in_maps(x, w_qkv, b_qkv, w_out, b_out, ln_w, ln_b):
    bf = ml_dtypes.bfloat16
    s_q = 1.0 / np.sqrt(HD)
    wq = w_qkv[0:D, :]
    wk = w_qkv[D : 2 * D, :]
    wv = w_qkv[2 * D : 3 * D, :]
    bq, bk, bvv = b_qkv[0:D], b_qkv[D : 2 * D], b_qkv[2 * D : 3 * D]
    woutT_full = np.ascontiguousarray(w_out.T) * 16.0  # undo the 1/16 in recip rows

    in_maps = []
    for c in range(N_CORES):
        b, g = c // 2, c % 2
        rows = slice(g * 512, (g + 1) * 512)
        wqg = (wq[rows, :] * s_q).astype(bf)
        wkg = wk[rows, :].astype(bf)
        wqkT = np.ascontiguousarray(np.concatenate([wqg, wkg], axis=0).T.astype(bf))
        xb = x[b]
        half = slice(g * SH, g * SH + SH)
        in_maps.append(
            {
                "xT": np.ascontiguousarray(xb.T.astype(bf)),
                "xr": np.ascontiguousarray(xb[half, :]).astype(np.float32),
                "wqkT": wqkT,
                "wvT": np.ascontiguousarray(wv[rows, :].T.astype(bf)),
                "woutT": np.ascontiguousarray(woutT_full[rows, :].astype(bf)),
                "bqk": np.concatenate([bq[rows] * s_q, bk[rows]]).astype(np.float32),
                "bv": bvv[rows].astype(np.float32),
                "bo": (b_out * 0.5).astype(np.float32),
                "lnw": ln_w.astype(np.float32),
                "lnb": ln_b.astype(np.float32),
            }
        )
    return in_maps


def _assemble(results):
    y = np.empty((B, S, D), dtype=np.float32)
    attn = np.empty((B, S, S), dtype=np.float32)
    for b in range(B):
        even, odd = results[2 * b], results[2 * b + 1]
        y[b, 0:SH, :] = even["y"]
        y[b, SH:S, :] = odd["y"]
        # chunked RS: each half-collective scatters its chunk across the pair
        ev, od = even["attn"].astype(np.float32), odd["attn"].astype(np.float32)
        at = np.concatenate([ev[0:256], od[0:256], ev[256:512], od[256:512]], axis=0)
        attn[b] = at.T
    return y, attn


def _flags(b_qkv, b_out, ln_w, ln_b):
    ln_affine = not (np.all(ln_w == 1.0) and np.all(ln_b == 0.0))
    bv_zero = bool(np.all(b_qkv[2 * D : 3 * D] == 0.0))
    bo_zero = bool(np.all(b_out == 0.0))
    return (ln_affine, bv_zero, bo_zero)


def kernel(x, w_qkv, b_qkv, w_out, b_out, ln_w, ln_b, _trace=False):
    from concourse.bass_utils import run_bass_kernel_spmd

    x = np.asarray(x, dtype=np.float32)
    w_qkv = np.asarray(w_qkv, dtype=np.float32)
    b_qkv = np.asarray(b_qkv, dtype=np.float32)
    w_out = np.asarray(w_out, dtype=np.float32)
    b_out = np.asarray(b_out, dtype=np.float32)
    ln_w = np.asarray(ln_w, dtype=np.float32)
    ln_b = np.asarray(ln_b, dtype=np.float32)

    nc = _get_nc(_flags(b_qkv, b_out, ln_w, ln_b))
    in_maps = _prep_in_maps(x, w_qkv, b_qkv, w_out, b_out, ln_w, ln_b)
    res = run_bass_kernel_spmd(nc, in_maps, core_ids=list(range(N_CORES)), trace=_trace)
    out = _assemble(res.results)
    if _trace:
        kernel.last_exec_time_ns = res.exec_time_ns
    return out


# ---- simulation entry for development (not used by the harness) ----
def simulate(x, w_qkv, b_qkv, w_out, b_out, ln_w, ln_b):
    from concourse import bass_interp

    nc = _get_nc(_flags(np.asarray(b_qkv), np.asarray(b_out),
                        np.asarray(ln_w), np.asarray(ln_b)))
    in_maps = _prep_in_maps(
        np.asarray(x, np.float32), np.asarray(w_qkv, np.float32),
        np.asarray(b_qkv, np.float32), np.asarray(w_out, np.float32),
        np.asarray(b_out, np.float32), np.asarray(ln_w, np.float32),
        np.asarray(ln_b, np.float32),
    )
    sim = bass_interp.MultiCoreSim(nc, N_CORES)
    for i in range(N_CORES):
        for k, vv in in_maps[i].items():
            sim.cores[i].tensor(k)[:] = vv
    sim.simulate()
    results = [
        {k: np.array(sim.cores[i].mem_tensor(k)) for k in ("y", "attn")}
        for i in range(N_CORES)
    ]
    return _assemble(results)
